# revision 1
# baseline (speedup 1.0000x reference)
"""Trainium2 Bass kernel for nn_DataEmbedding (linear embed + positional + GCN).

out[b,n,t,:] = x[b,n,t,:] @ W_lin + b_lin + pe[t,:] + gcn(emb_table)[n,:]

Sharding: graph-partitioned by destination node. Core k owns nodes
[625k, 625(k+1)) and produces the output shard out[:, 625k:625(k+1), :, :].
No collectives. Host does index/layout prep only (edge sort by destination,
padding, gather-map construction); all floating-point math runs on device.

GCN message passing: g = D^-1/2 (emb @ W_gcn) is stored to DRAM in bf16;
per 128-edge chunk the source rows are gathered (indirect DMA, round-robin
over 4 SWDGE queues), a one-hot selection matrix S = (iota==dst_local)*w is
built on DVE, and the PE accumulates S^T @ M into PSUM per 125-node block.
"""

import numpy as np
import ml_dtypes

import concourse.bacc as bacc
import concourse.bass as bass
import concourse.mybir as mybir
from concourse.bass_utils import run_bass_kernel_spmd
from concourse.tile import TileContext

# problem constants (hardcoded per contract)
B, N, T, CIN, D, E = 8, 5000, 12, 3, 256, 160000
NCORES = 8
NPC = N // NCORES        # nodes per core = 625
BLK = 125                # destination nodes per PSUM block
NBLK = NPC // BLK        # blocks per core = 5
W = 4                    # 128-edge sub-chunks per S-build group
CHUNK = 128 * W          # edges per chunk group = 512
NT = (N + 127) // 128    # global 128-node tiles = 40
KX = 3 * T + 2           # main matmul contraction: (t,c) rows + pe + bias = 38

f32 = mybir.dt.float32
b16 = mybir.dt.bfloat16
i32 = mybir.dt.int32

_QUEUES = ["qPoolDynamic", "qPoolDynamic1", "qPoolDynamic2", "qPoolDynamic3"]


def _pe_table() -> np.ndarray:
    pos = np.arange(T, dtype=np.float32)[:, None]
    div = np.exp(np.arange(0, D, 2, dtype=np.float32) * (-np.log(10000.0) / D))
    pe = np.zeros((T, D), dtype=np.float32)
    pe[:, 0::2] = np.sin(pos * div)
    pe[:, 1::2] = np.cos(pos * div)
    return pe


def _prep(x, edge_index, weights, W_lin, b_lin):
    """Host-side sharding/layout prep. Index manipulation and data movement
    only — no arithmetic on float input values."""
    ei = np.asarray(edge_index)
    row2 = np.concatenate([ei[0], np.arange(N)]).astype(np.int32)
    col2 = np.concatenate([ei[1], np.arange(N)]).astype(np.int32)
    w2 = np.concatenate(
        [np.asarray(weights, dtype=np.float32), np.ones(N, dtype=np.float32)]
    )
    order = np.argsort(col2, kind="stable")
    row_s, col_s, w_s = row2[order], col2[order], w2[order]

    # padded per-node weight matrix for on-device degree = row-sum
    starts = np.searchsorted(col_s, np.arange(N)).astype(np.int64)
    cnt = np.bincount(col2, minlength=N)
    L = int(max(8, ((cnt.max() + 7) // 8) * 8))
    NP = NT * 128  # 5120, node count padded to full 128-tiles
    wpad = np.zeros((NP, L), dtype=np.float32)
    offs = np.arange(len(col_s), dtype=np.int64) - starts[col_s]
    wpad[col_s, offs] = w_s
    wpad[N:, 0] = 1.0  # pad rows: deg=1 so dinv stays finite
    wpad_pm = np.ascontiguousarray(
        wpad.reshape(NT, 128, L).transpose(1, 0, 2).reshape(128, NT * L)
    )

    # per-(core, block) sorted edge runs, padded to CHUNK multiples
    blk_lo = np.arange(NCORES * NBLK) * BLK
    seg_s = np.searchsorted(col_s, blk_lo)
    seg_e = np.searchsorted(col_s, blk_lo + BLK)
    CH = int(np.ceil((seg_e - seg_s).max() / CHUNK))

    # main-matmul rhs [KX, T*D]: rows 3t+c carry W_lin[c] in the t-block of
    # columns (block "diagonal"), row 36 = positional encoding, row 37 = b_lin
    pe = _pe_table()
    rhs38 = np.zeros((KX, T * D), dtype=np.float32)
    for t in range(T):
        for c in range(CIN):
            rhs38[3 * t + c, t * D : (t + 1) * D] = np.asarray(W_lin, np.float32)[c]
    rhs38[36] = pe.reshape(-1)
    rhs38[37] = np.tile(np.asarray(b_lin, dtype=np.float32), T)

    xa = np.asarray(x, dtype=np.float32)
    SW = CHUNK // 16  # int16 idx columns per chunk (wrapped in 16 partitions)
    per_core = []
    for k in range(NCORES):
        gidx = np.zeros((16, NBLK * CH * SW), dtype=np.int16)
        wsv = np.zeros((128, NBLK * CH * W), dtype=ml_dtypes.bfloat16)
        cloc = np.full((128, NBLK * CH * W), float(BLK), dtype=np.float32)
        for blk in range(NBLK):
            gi = NBLK * k + blk
            s, e = int(seg_s[gi]), int(seg_e[gi])
            n = e - s
            assert n <= CH * CHUNK
            pad = CH * CHUNK - n
            # g_d rows live in SBUF-native order: node n at row (n%128)*NT+n//128
            src = row_s[s:e].astype(np.int64)
            gi_p = np.pad((src % 128) * NT + src // 128, (0, pad))
            wv_p = np.pad(w_s[s:e], (0, pad))
            cl_p = np.pad(
                (col_s[s:e] - blk_lo[gi]).astype(np.float32),
                (0, pad),
                constant_values=float(BLK),
            )
            # dma_gather idx wrap: chunk slot i=(j*128+p) at [i%16, i//16]
            c0i = blk * CH * SW
            gidx[:, c0i : c0i + CH * SW] = (
                gi_p.reshape(CH, SW, 16).transpose(2, 0, 1).reshape(16, CH * SW)
            )
            # S-build slot (ch, j, p) -> column blk*CH*W + ch*W + j, partition p
            c0 = blk * CH * W
            wsv[:, c0 : c0 + CH * W] = (
                wv_p.reshape(CH, W, 128)
                .transpose(2, 0, 1)
                .reshape(128, CH * W)
                .astype(ml_dtypes.bfloat16)
            )
            cloc[:, c0 : c0 + CH * W] = (
                cl_p.reshape(CH, W, 128).transpose(2, 0, 1).reshape(128, CH * W)
            )

        # x in matmul-ready lhsT layout: [NBLK, KX, B*BLK] — K rows are
        # (t,c) pairs then two ones-rows (pe, bias); all 8 batches along free
        xs = xa[:, k * NPC : (k + 1) * NPC].reshape(B, NBLK, BLK, T, CIN)
        x38 = np.ones((NBLK, KX, B, BLK), dtype=np.float32)
        # [B, NBLK, BLK, T, CIN] -> [NBLK, T, CIN, B, BLK]
        x38[:, : 3 * T] = xs.transpose(1, 3, 4, 0, 2).reshape(NBLK, 3 * T, B, BLK)
        per_core.append(
            {
                "gidx": np.ascontiguousarray(np.tile(gidx, (8, 1))),
                "ws": wsv,
                "cloc": cloc,
                "x38": np.ascontiguousarray(x38.reshape(NBLK, KX, B * BLK)),
                "wpad_loc": np.ascontiguousarray(
                    wpad[k * NPC : (k + 1) * NPC]
                    .reshape(NBLK, BLK, L)
                    .transpose(1, 0, 2)
                    .reshape(BLK, NBLK * L)
                ),
            }
        )
    return per_core, wpad_pm, rhs38, CH, L


_KERNEL_CACHE: dict = {}


def _build_kernel(CH: int, L: int):
    key = (CH, L)
    if key in _KERNEL_CACHE:
        return _KERNEL_CACHE[key]

    nc = bacc.Bacc(num_swdge_queues=4)
    x38_d = nc.declare_dram_parameter("x38", [NBLK, KX, B * BLK], f32, isOutput=False)
    SW = CHUNK // 16
    gidx_d = nc.declare_dram_parameter(
        "gidx", [128, NBLK * CH * SW], mybir.dt.int16, isOutput=False
    )
    ws_d = nc.declare_dram_parameter("ws", [128, NBLK * CH * W], b16, isOutput=False)
    cloc_d = nc.declare_dram_parameter("cloc", [128, NBLK * CH * W], f32, isOutput=False)
    wpad_d = nc.declare_dram_parameter("wpad", [128, NT * L], f32, isOutput=False)
    wploc_d = nc.declare_dram_parameter("wpad_loc", [BLK, NBLK * L], f32, isOutput=False)
    embT_d = nc.declare_dram_parameter("embT", [D, N], f32, isOutput=False)
    wg_d = nc.declare_dram_parameter("W_gcn", [D, D], f32, isOutput=False)
    bg_d = nc.declare_dram_parameter("b_gcn", [1, D], f32, isOutput=False)
    rhs38_d = nc.declare_dram_parameter("rhs38", [KX, T * D], f32, isOutput=False)
    out_d = nc.declare_dram_parameter("out", [B, NPC, T, D], f32, isOutput=True)
    g_d = nc.dram_tensor("g_scaled", [NT * 128, D], b16)

    def last_inst():
        return list(nc.inst_map.values())[-1]

    with TileContext(nc) as tc:
        with tc.tile_pool(name="const", bufs=1) as cp:
            # iota row pattern 0..BLK-1 repeated W times, as f32
            iota_i = cp.tile([128, W * BLK], i32)
            nc.gpsimd.iota(iota_i[:], pattern=[[0, W], [1, BLK]], base=0,
                           channel_multiplier=0)
            iota_f = cp.tile([128, W * BLK], f32)
            nc.vector.tensor_copy(iota_f[:], iota_i[:])
            ones_row = cp.tile([1, BLK], f32)
            nc.vector.memset(ones_row[:], 1.0)

            rhs38 = cp.tile([KX, T * D], f32)
            nc.scalar.dma_start(out=rhs38[:], in_=rhs38_d[:])
            wg0 = cp.tile([128, D], f32)
            wg1 = cp.tile([128, D], f32)
            nc.scalar.dma_start(out=wg0[:], in_=wg_d[0:128, :])
            nc.scalar.dma_start(out=wg1[:], in_=wg_d[128:256, :])
            bg_row = cp.tile([1, D], f32)
            nc.scalar.dma_start(out=bg_row[:], in_=bg_d[:])

            gidx = cp.tile([128, NBLK * CH * SW], mybir.dt.int16)
            wsv = cp.tile([128, NBLK * CH * W], b16)
            cloc = cp.tile([128, NBLK * CH * W], f32)
            nc.scalar.dma_start(out=gidx[:], in_=gidx_d[:])
            nc.scalar.dma_start(out=wsv[:], in_=ws_d[:])
            nc.scalar.dma_start(out=cloc[:], in_=cloc_d[:])

            dinv_all = cp.tile([128, NT], f32)
            dinv_loc = cp.tile([BLK, NBLK], f32)
            ve_all = cp.tile([BLK, NBLK * D], f32)
            b_rep = cp.tile([BLK, D], f32)
            w_all = cp.tile([128, NT * L], f32)
            wl_all = cp.tile([BLK, NBLK * L], f32)
            eT0 = cp.tile([128, N], f32)
            eT1 = cp.tile([128, N], f32)
            g_all = cp.tile([128, NT * D], b16)
            nc.scalar.dma_start(out=w_all[:], in_=wpad_d[:])
            nc.scalar.dma_start(out=wl_all[:], in_=wploc_d[:])
            nc.scalar.dma_start(out=eT0[:], in_=embT_d[0:128, :])
            nc.scalar.dma_start(out=eT1[:], in_=embT_d[128:256, :])

            # ---- phase A: degrees -> dinv ----
            with (
                tc.tile_pool(name="pA", bufs=4) as pA,
                tc.tile_pool(name="ppA", bufs=2, space="PSUM") as ppA,
            ):
                for j in range(NT):
                    deg = pA.tile([128, 1], f32, tag="deg")
                    nc.vector.reduce_sum(
                        out=deg[:],
                        in_=w_all[:, j * L : (j + 1) * L],
                        axis=mybir.AxisListType.X,
                    )
                    rec = pA.tile([128, 1], f32, tag="rec")
                    nc.vector.reciprocal(rec[:], deg[:])
                    nc.scalar.sqrt(dinv_all[:, j : j + 1], rec[:])
                for blk in range(NBLK):
                    deg = pA.tile([BLK, 1], f32, tag="degl")
                    nc.vector.reduce_sum(
                        out=deg[:],
                        in_=wl_all[:, blk * L : (blk + 1) * L],
                        axis=mybir.AxisListType.X,
                    )
                    rec = pA.tile([BLK, 1], f32, tag="recl")
                    nc.vector.reciprocal(rec[:], deg[:])
                    nc.scalar.sqrt(dinv_loc[:, blk : blk + 1], rec[:])

                # ---- phase B: g = dinv * (emb @ W_gcn) -> one DRAM write ----
                nc.vector.memset(g_all[:, (NT - 1) * D :], 0.0)
                for j in range(NT):
                    cols = min(128, N - j * 128)
                    hg = ppA.tile([128, D], f32, space="PSUM", tag="hg")
                    nc.tensor.matmul(
                        hg[:cols, :],
                        lhsT=eT0[:, j * 128 : j * 128 + cols],
                        rhs=wg0[:],
                        start=True,
                        stop=False,
                    )
                    nc.tensor.matmul(
                        hg[:cols, :],
                        lhsT=eT1[:, j * 128 : j * 128 + cols],
                        rhs=wg1[:],
                        start=False,
                        stop=True,
                    )
                    nc.vector.tensor_scalar_mul(
                        g_all[:cols, j * D : (j + 1) * D],
                        hg[:cols, :],
                        dinv_all[:cols, j : j + 1],
                    )
                nc.sync.dma_start(out=g_d[:].rearrange("(p a) d -> p a d", p=128),
                                  in_=g_all[:].rearrange("p (a d) -> p a d", d=D))
                # b_rep = ones(125,1) @ b_gcn(1,256)
                br = ppA.tile([BLK, D], f32, space="PSUM", tag="hg")
                nc.tensor.matmul(
                    br[:], lhsT=ones_row[0:1, :], rhs=bg_row[0:1, :], start=True, stop=True
                )
                nc.vector.tensor_copy(b_rep[:], br[:])

            # ---- phase C: per block, GCN scatter-matmul then main output ----
            with (
                tc.tile_pool(name="pC", bufs=4) as pC,
                tc.tile_pool(name="xload", bufs=2) as xload,
                tc.tile_pool(name="vps", bufs=2, space="PSUM") as vps,
                tc.tile_pool(name="mps", bufs=2, space="PSUM") as mps,
                tc.tile_pool(name="outp", bufs=3) as outp,
            ):
                qi = 0
                for blk in range(NBLK):
                    vp = vps.tile([BLK, D], f32, space="PSUM", tag="vp")
                    for ch in range(CH):
                        c0 = (blk * CH + ch) * W
                        ci = (blk * CH + ch) * SW
                        M = pC.tile([128, W * D], b16, tag="M")
                        nc.gpsimd.dma_gather(
                            out_ap=M[:].rearrange("p (c d) -> p c d", d=D),
                            in_ap=g_d[:],
                            idxs_ap=gidx[:, ci : ci + SW],
                            num_idxs=CHUNK,
                            num_idxs_reg=CHUNK,
                            elem_size=D,
                            single_packet=False,
                            queue_num=qi % 4,
                        )
                        qi += 1
                        # S = (iota == cloc) * ws ; cloc/ws broadcast along BLK
                        S0 = pC.tile([128, W * BLK], b16, tag="S")
                        nc.vector.tensor_tensor(
                            out=S0[:],
                            in0=iota_f[:],
                            in1=cloc[:, c0 : c0 + W, None].to_broadcast(
                                [128, W, BLK]
                            ),
                            op=mybir.AluOpType.is_equal,
                        )
                        nc.vector.tensor_tensor(
                            out=S0[:],
                            in0=S0[:],
                            in1=wsv[:, c0 : c0 + W, None].to_broadcast([128, W, BLK]),
                            op=mybir.AluOpType.mult,
                        )
                        for j in range(W):
                            nc.tensor.matmul(
                                vp[:],
                                lhsT=S0[:, j * BLK : (j + 1) * BLK],
                                rhs=M[:, j * D : (j + 1) * D],
                                start=(ch == 0 and j == 0),
                                stop=(ch == CH - 1 and j == W - 1),
                            )
                    ve = ve_all[:, blk * D : (blk + 1) * D]
                    nc.vector.tensor_scalar_mul(ve, vp[:], dinv_loc[:, blk : blk + 1])
                    nc.vector.tensor_add(ve, ve, b_rep[:])
                    ve3 = ve.rearrange("p d -> p () d").to_broadcast([BLK, T // 2, D])

                    x38t = xload.tile([KX, B * BLK], f32, tag="x")
                    nc.scalar.dma_start(out=x38t[:], in_=x38_d[blk])
                    for b in range(B):
                        lhsT = x38t[:, b * BLK : (b + 1) * BLK]
                        osb = outp.tile([BLK, T * D], f32, tag="osb")
                        for half in range(2):
                            mp = mps.tile([BLK, 3 * 512], f32, space="PSUM", tag="mp")
                            for i in range(3):
                                tp = half * 3 + i  # t-pair index
                                nc.tensor.matmul(
                                    mp[:, i * 512 : (i + 1) * 512],
                                    lhsT=lhsT,
                                    rhs=rhs38[:, tp * 512 : (tp + 1) * 512],
                                    start=True,
                                    stop=True,
                                )
                            nc.vector.tensor_tensor(
                                out=osb[:, half * 1536 : (half + 1) * 1536].rearrange(
                                    "p (t d) -> p t d", d=D
                                ),
                                in0=mp[:].rearrange("p (t d) -> p t d", d=D),
                                in1=ve3,
                                op=mybir.AluOpType.add,
                            )
                        nc.sync.dma_start(
                            out=out_d[b, blk * BLK : (blk + 1) * BLK].rearrange(
                                "p t d -> p (t d)"
                            ),
                            in_=osb[:],
                        )

    nc.finalize()  # run bacc passes (reg alloc, TRN2 sync-wait splitting)
    _KERNEL_CACHE[key] = nc
    return nc


LAST_RESULTS = None  # BassKernelResults of the most recent run (for profiling)


def kernel(x, x_mark, edge_index, weights, W_lin, b_lin, emb_table, W_gcn, b_gcn):
    global LAST_RESULTS
    per_core, wpad, rhs38, CH, L = _prep(x, edge_index, weights, W_lin, b_lin)
    nc = _build_kernel(CH, L)
    embT = np.ascontiguousarray(np.asarray(emb_table, dtype=np.float32).T)
    shared = {
        "wpad": wpad,
        "embT": embT,
        "W_gcn": np.asarray(W_gcn, dtype=np.float32),
        "b_gcn": np.asarray(b_gcn, dtype=np.float32).reshape(1, D),
        "rhs38": rhs38,
    }
    in_maps = [{**shared, **pc} for pc in per_core]
    res = run_bass_kernel_spmd(nc, in_maps, list(range(NCORES)))
    LAST_RESULTS = res
    shards = [res.results[k]["out"] for k in range(NCORES)]
    return np.concatenate(shards, axis=1)



# revision 3
# speedup vs baseline: 2.0300x; 2.0300x over previous
"""Trainium2 Bass kernel for nn_DataEmbedding (linear embed + positional + GCN).

out[b,n,t,:] = x[b,n,t,:] @ W_lin + b_lin + pe[t,:] + gcn(emb_table)[n,:]

Sharding: graph-partitioned by destination node. Core k owns nodes
[625k, 625(k+1)) and produces the output shard out[:, 625k:625(k+1), :, :].
No collectives. Host does index/layout prep only (edge scatter into a dense
adjacency, padding, bf16 casts); all floating-point math runs on device.

GCN message passing is a dense matmul: the host scatters raw edge weights
into A[src, dst] (bf16, [5120 x 625] per core, ~0.6% dense), and the device
computes vp = A^T @ g with 40 accumulating 128-contraction matmuls per
125-node block, where g = D^-1/2 (emb @ W_gcn). Destination normalization
and bias fold into the per-block ve finalize. This replaces the per-edge
indirect-DMA gather + one-hot scatter (which was descriptor-bound).

Main output: per (block, batch) a [38 x 125] bf16 lhsT (x rows + ones rows
for pe/bias) hits a [38 x 3072] rhs; PSUM is evacuated with the ve add,
split between the Vector and Scalar engines, and written to DRAM in bf16.
"""

import numpy as np
import ml_dtypes

import concourse.bacc as bacc
import concourse.bass as bass
import concourse.mybir as mybir
from concourse.bass_utils import run_bass_kernel_spmd
from concourse.tile import TileContext

# problem constants (hardcoded per contract)
B, N, T, CIN, D, E = 8, 5000, 12, 3, 256, 160000
NCORES = 8
NPC = N // NCORES        # nodes per core = 625
BLK = 125                # destination nodes per PSUM block
NBLK = NPC // BLK        # blocks per core = 5
NT = (N + 127) // 128    # global 128-node source tiles = 40
NP = NT * 128            # padded source count = 5120
KX = 3 * T + 2           # main matmul contraction: (t,c) rows + pe + bias = 38
TP = T * D // 512        # 512-col tiles across (t,d) = 6
HALF = 3 * 512           # free elems per evacuation half = 1536

f32 = mybir.dt.float32
b16 = mybir.dt.bfloat16

bf = ml_dtypes.bfloat16


def _pe_table() -> np.ndarray:
    pos = np.arange(T, dtype=np.float32)[:, None]
    div = np.exp(np.arange(0, D, 2, dtype=np.float32) * (-np.log(10000.0) / D))
    pe = np.zeros((T, D), dtype=np.float32)
    pe[:, 0::2] = np.sin(pos * div)
    pe[:, 1::2] = np.cos(pos * div)
    return pe


def _prep(x, edge_index, weights, W_lin, b_lin):
    """Host-side sharding/layout prep: edge scatter, padding, bf16 casts."""
    ei = np.asarray(edge_index)
    row2 = np.concatenate([ei[0], np.arange(N)]).astype(np.int64)  # src
    col2 = np.concatenate([ei[1], np.arange(N)]).astype(np.int64)  # dst
    w2 = np.concatenate(
        [np.asarray(weights, dtype=np.float32), np.ones(N, dtype=np.float32)]
    )

    # dense adjacency A[src, dst] of raw weights (self-loops w=1 included);
    # duplicate (src,dst) edges accumulate, matching segment_sum semantics
    A = np.zeros((NP, N), dtype=np.float32)
    np.add.at(A, (row2, col2), w2)

    # padded per-node incoming-weight matrix for on-device degree = row-sum
    order = np.argsort(col2, kind="stable")
    col_s, w_s = col2[order], w2[order]
    starts = np.searchsorted(col_s, np.arange(N)).astype(np.int64)
    cnt = np.bincount(col2, minlength=N)
    L = int(max(8, ((cnt.max() + 7) // 8) * 8))
    wpad = np.zeros((NP, L), dtype=np.float32)
    offs = np.arange(len(col_s), dtype=np.int64) - starts[col_s]
    wpad[col_s, offs] = w_s
    wpad[N:, 0] = 1.0  # pad rows: deg=1 so dinv stays finite
    wpad_pm = np.ascontiguousarray(
        wpad.reshape(NT, 128, L).transpose(1, 0, 2).reshape(128, NT * L)
    )

    # main-matmul rhs [KX, T*D]: rows 3t+c carry W_lin[c] in the t-block of
    # columns, row 36 = positional encoding, row 37 = b_lin tiled
    pe = _pe_table()
    rhs38 = np.zeros((KX, T * D), dtype=np.float32)
    for t in range(T):
        for c in range(CIN):
            rhs38[3 * t + c, t * D : (t + 1) * D] = np.asarray(W_lin, np.float32)[c]
    rhs38[36] = pe.reshape(-1)
    rhs38[37] = np.tile(np.asarray(b_lin, dtype=np.float32), T)

    xa = np.asarray(x, dtype=np.float32)
    per_core = []
    for k in range(NCORES):
        # A tiles in matmul lhsT layout: [128 src-partition,
        # (blk*NT + j)*BLK + dst-local] bf16
        Ak = A[:, k * NPC : (k + 1) * NPC]
        A_sb = np.ascontiguousarray(
            Ak.reshape(NT, 128, NBLK, BLK)
            .transpose(1, 2, 0, 3)
            .reshape(128, NBLK * NT * BLK)
            .astype(bf)
        )
        # x in matmul-ready lhsT layout [KX, NBLK*B*BLK] bf16: rows are
        # (t,c) pairs then two ones-rows (pe, bias)
        xs = xa[:, k * NPC : (k + 1) * NPC].reshape(B, NBLK, BLK, T, CIN)
        x38 = np.ones((KX, NBLK, B, BLK), dtype=np.float32)
        x38[: 3 * T] = xs.transpose(3, 4, 1, 0, 2).reshape(3 * T, NBLK, B, BLK)
        per_core.append(
            {
                "A": A_sb,
                "x38": np.ascontiguousarray(x38.reshape(KX, NBLK * B * BLK)).astype(bf),
                "wpad_loc": np.ascontiguousarray(
                    wpad[k * NPC : (k + 1) * NPC]
                    .reshape(NBLK, BLK, L)
                    .transpose(1, 0, 2)
                    .reshape(BLK, NBLK * L)
                ),
            }
        )
    return per_core, wpad_pm, rhs38.astype(bf), L


_KERNEL_CACHE: dict = {}


def _build_kernel(L: int):
    if L in _KERNEL_CACHE:
        return _KERNEL_CACHE[L]

    nc = bacc.Bacc()
    x38_d = nc.declare_dram_parameter("x38", [KX, NBLK * B * BLK], b16, isOutput=False)
    A_d = nc.declare_dram_parameter("A", [128, NBLK * NT * BLK], b16, isOutput=False)
    wpad_d = nc.declare_dram_parameter("wpad", [128, NT * L], f32, isOutput=False)
    wploc_d = nc.declare_dram_parameter("wpad_loc", [BLK, NBLK * L], f32, isOutput=False)
    embT_d = nc.declare_dram_parameter("embT", [D, N], b16, isOutput=False)
    wg_d = nc.declare_dram_parameter("W_gcn", [D, D], b16, isOutput=False)
    bg_d = nc.declare_dram_parameter("b_gcn", [1, D], f32, isOutput=False)
    rhs38_d = nc.declare_dram_parameter("rhs38", [KX, T * D], b16, isOutput=False)
    out_d = nc.declare_dram_parameter("out", [B, NPC, T, D], b16, isOutput=True)

    with TileContext(nc) as tc:
        with tc.tile_pool(name="const", bufs=1) as cp:
            ones_row = cp.tile([1, BLK], f32)
            nc.vector.memset(ones_row[:], 1.0)

            w_all = cp.tile([128, NT * L], f32)
            wl_all = cp.tile([BLK, NBLK * L], f32)
            nc.scalar.dma_start(out=w_all[:], in_=wpad_d[:])
            nc.scalar.dma_start(out=wl_all[:], in_=wploc_d[:])

            wg0 = cp.tile([128, D], b16)
            wg1 = cp.tile([128, D], b16)
            nc.scalar.dma_start(out=wg0[:], in_=wg_d[0:128, :])
            nc.scalar.dma_start(out=wg1[:], in_=wg_d[128:256, :])
            bg_row = cp.tile([1, D], f32)
            nc.scalar.dma_start(out=bg_row[:], in_=bg_d[:])

            rhs38 = cp.tile([KX, T * D], b16)
            nc.scalar.dma_start(out=rhs38[:], in_=rhs38_d[:])
            x38 = cp.tile([KX, NBLK * B * BLK], b16)
            nc.scalar.dma_start(out=x38[:], in_=x38_d[:])

            A_sb = []
            for blk in range(NBLK):
                a = cp.tile([128, NT * BLK], b16, tag=f"A{blk}")
                nc.scalar.dma_start(
                    out=a[:], in_=A_d[:, blk * NT * BLK : (blk + 1) * NT * BLK]
                )
                A_sb.append(a)

            dinv_all = cp.tile([128, NT], f32)
            dinv_loc = cp.tile([BLK, NBLK], f32)
            g_all = cp.tile([128, NT * D], b16)
            b_rep = cp.tile([BLK, D], f32)

            # ---- phase A: degrees -> dinv (batched reduce + rsqrt) ----
            with (
                tc.tile_pool(name="pA", bufs=2) as pA,
                tc.tile_pool(name="ppA", bufs=2, space="PSUM") as ppA,
                tc.tile_pool(name="emb", bufs=1) as ep,
            ):
                eT0 = ep.tile([128, N], b16)
                eT1 = ep.tile([128, N], b16)
                nc.scalar.dma_start(out=eT0[:], in_=embT_d[0:128, :])
                nc.scalar.dma_start(out=eT1[:], in_=embT_d[128:256, :])

                dega = pA.tile([128, NT], f32, tag="dega")
                nc.vector.reduce_sum(
                    out=dega[:],
                    in_=w_all[:].rearrange("p (j l) -> p j l", l=L),
                    axis=mybir.AxisListType.X,
                )
                reca = pA.tile([128, NT], f32, tag="reca")
                nc.vector.reciprocal(reca[:], dega[:])
                nc.scalar.sqrt(dinv_all[:], reca[:])

                degl = pA.tile([BLK, NBLK], f32, tag="degl")
                nc.vector.reduce_sum(
                    out=degl[:],
                    in_=wl_all[:].rearrange("p (j l) -> p j l", l=L),
                    axis=mybir.AxisListType.X,
                )
                recl = pA.tile([BLK, NBLK], f32, tag="recl")
                nc.vector.reciprocal(recl[:], degl[:])
                nc.scalar.sqrt(dinv_loc[:], recl[:])

                # ---- phase B: g = dinv * (emb @ W_gcn), bf16 in SBUF ----
                nc.vector.memset(g_all[:, (NT - 1) * D :], 0.0)
                for j in range(NT):
                    cols = min(128, N - j * 128)
                    hg = ppA.tile([128, D], f32, space="PSUM", tag="hg")
                    nc.tensor.matmul(
                        hg[:cols, :],
                        lhsT=eT0[:, j * 128 : j * 128 + cols],
                        rhs=wg0[:],
                        start=True,
                        stop=False,
                    )
                    nc.tensor.matmul(
                        hg[:cols, :],
                        lhsT=eT1[:, j * 128 : j * 128 + cols],
                        rhs=wg1[:],
                        start=False,
                        stop=True,
                    )
                    nc.scalar.mul(
                        g_all[:cols, j * D : (j + 1) * D],
                        hg[:cols, :],
                        dinv_all[:cols, j : j + 1],
                    )
                # b_rep = ones(125,1) @ b_gcn(1,256)
                br = ppA.tile([BLK, D], f32, space="PSUM", tag="hg")
                nc.tensor.matmul(
                    br[:], lhsT=ones_row[0:1, :], rhs=bg_row[0:1, :],
                    start=True, stop=True,
                )
                nc.vector.tensor_copy(b_rep[:], br[:])

            # ---- phase C: per block, dense-A GCN matmul then main output ----
            with (
                tc.tile_pool(name="vef", bufs=2) as vef,
                tc.tile_pool(name="veb", bufs=2) as veb,
                tc.tile_pool(name="vps", bufs=2, space="PSUM") as vps,
                tc.tile_pool(name="mps", bufs=2, space="PSUM") as mps,
                tc.tile_pool(name="outp", bufs=3) as outp,
            ):
                for blk in range(NBLK):
                    # vp[dst, :] = sum_src A[src, dst] * g[src, :]
                    vp = vps.tile([BLK, D], f32, space="PSUM", tag="vp")
                    for j in range(NT):
                        nc.tensor.matmul(
                            vp[:],
                            lhsT=A_sb[blk][:, j * BLK : (j + 1) * BLK],
                            rhs=g_all[:, j * D : (j + 1) * D],
                            start=(j == 0),
                            stop=(j == NT - 1),
                        )
                    ve = vef.tile([BLK, D], f32, tag="ve")
                    nc.scalar.mul(ve[:], vp[:], dinv_loc[:, blk : blk + 1])
                    # veps = (ve + b_gcn) tiled over the 6 t-slots of a half
                    veps = veb.tile([BLK, HALF], b16, tag="veps")
                    nc.vector.tensor_tensor(
                        out=veps[:].rearrange("p (t d) -> p t d", d=D),
                        in0=ve[:].rearrange("p d -> p () d").to_broadcast(
                            [BLK, HALF // D, D]
                        ),
                        in1=b_rep[:].rearrange("p d -> p () d").to_broadcast(
                            [BLK, HALF // D, D]
                        ),
                        op=mybir.AluOpType.add,
                    )

                    for b in range(B):
                        lhsT = x38[:, (blk * B + b) * BLK : (blk * B + b + 1) * BLK]
                        osb = outp.tile([BLK, T * D], b16, tag="osb")
                        for half in range(2):
                            mp = mps.tile([BLK, HALF], f32, space="PSUM", tag="mp")
                            for i in range(3):
                                tp = half * 3 + i
                                nc.tensor.matmul(
                                    mp[:, i * 512 : (i + 1) * 512],
                                    lhsT=lhsT,
                                    rhs=rhs38[:, tp * 512 : (tp + 1) * 512],
                                    start=True,
                                    stop=True,
                                )
                            dst = osb[:, half * HALF : (half + 1) * HALF]
                            h = (blk * B + b) * 2 + half
                            if h % 4 == 0:
                                # route a: DVE adds ve while evacuating PSUM
                                nc.vector.tensor_tensor(
                                    out=dst.rearrange("p (t d) -> p t d", d=D),
                                    in0=mp[:].rearrange("p (t d) -> p t d", d=D),
                                    in1=veps[:].rearrange("p (t d) -> p t d", d=D),
                                    op=mybir.AluOpType.add,
                                )
                            else:
                                # route b: ACT evacuates PSUM to bf16, DVE
                                # adds ve in-place at 2x (16-bit) rate
                                nc.scalar.copy(dst, mp[:])
                                nc.vector.tensor_tensor(
                                    out=dst,
                                    in0=dst,
                                    in1=veps[:],
                                    op=mybir.AluOpType.add,
                                )
                        nc.sync.dma_start(
                            out=out_d[b, blk * BLK : (blk + 1) * BLK].rearrange(
                                "p t d -> p (t d)"
                            ),
                            in_=osb[:],
                        )

    nc.finalize()  # run bacc passes (reg alloc, TRN2 sync-wait splitting)
    _KERNEL_CACHE[L] = nc
    return nc


LAST_RESULTS = None  # BassKernelResults of the most recent run (for profiling)


def kernel(x, x_mark, edge_index, weights, W_lin, b_lin, emb_table, W_gcn, b_gcn):
    global LAST_RESULTS
    per_core, wpad, rhs38, L = _prep(x, edge_index, weights, W_lin, b_lin)
    nc = _build_kernel(L)
    embT = np.ascontiguousarray(np.asarray(emb_table, dtype=np.float32).T.astype(bf))
    shared = {
        "wpad": wpad,
        "embT": embT,
        "W_gcn": np.asarray(W_gcn, dtype=np.float32).astype(bf),
        "b_gcn": np.asarray(b_gcn, dtype=np.float32).reshape(1, D),
        "rhs38": rhs38,
    }
    in_maps = [{**shared, **pc} for pc in per_core]
    res = run_bass_kernel_spmd(nc, in_maps, list(range(NCORES)))
    LAST_RESULTS = res
    shards = [np.asarray(res.results[k]["out"]) for k in range(NCORES)]
    return np.concatenate(shards, axis=1).astype(np.float32)
